# revision 22
# baseline (speedup 1.0000x reference)
# Trainium2 Bass kernel for nn_DiffNet.
#
# Math: the conv2(conv1(.)) meta-MLP is affine per element, so with
#   coef = (conv2_w @ conv1_w)[0]  (c0, c1, c2),
#   bc   = (conv2_w @ conv1_b)[0] + conv2_b[0],
#   scale = RATE / batch_num,
# each layer (W, b) of the reference reduces to
#   z  = vi @ W.T                      (pre-bias matmul)
#   vj = relu(z + b)
#   s  = rowsum(vi),  q = rowsum(vi^2)
#   out = (1 + C2*s) * vj + C1*z + (C0*q + Cb*s)
# with C* = scale * (c*, bc).  No [B, out, in] tensor is ever materialized.
#
# Sharding: data-parallel over batch (64 rows -> 8 rows/core), weights
# replicated per core, zero collectives.
#
# Device-side bias folding: PSUM holds P = vi' @ W.T + bhat, where inputs are
# represented as vi = vi' + m (m a constant row vector, m1 = 0) and
# bhat = b + m @ W.T, so P = z + b exactly.  Then
#   out' = alpha (.) relu(P) + C1*P + delta,   out = out' - C1*b,
# so the next layer's constant offset is m_next = -C1*b, folded on host into
# bhat_next, k_alpha, k_delta, and the q cross-term.
#
# Matmul operands are fp16 (4x PE rate vs fp32, half the HBM bytes);
# accumulation + epilogue stay fp32 (measured l2 rel err ~5e-4).
#
# Perf notes (from HW traces):
# - HWDGE descriptor-gen paces a queue at ~desc_size/20ns; per-partition
#   runs must be >=4KB, so all fp16 operands live in ONE [128, 7232] pack
#   (xt | w1 | w2 | w3) DMA'd in 4 column-slices on the sync queue while
#   pk1/pk8 ride the scalar queue.
# - PE HAM clock-gate: ~4us of warm-up matmuls on junk tiles first, so the
#   real matmuls run at 2.4GHz instead of 1.2.
# - Kernel tail pays ~115ns per semaphore reset: keep instruction count low
#   (fused delta reduction, single transpose-copy per boundary).

import numpy as np

RATE = 0.01
B, IN, H1, H2, OUT = 64, 1024, 512, 512, 256
NCORES = 8
BL = B // NCORES  # 8 rows per core
P128 = 128

# const columns in pk8: scalars, then per-layer [Cb, C0, 2C0] triples
C_C1, C_C2 = 0, 1
C_KA0 = 2    # 2,3,4 = k_alpha per layer
C_KD0 = 5    # 5,6,7 = k_delta per layer
C_ZERO = 8
C_TRI0 = 9   # 9..17: per-layer [Cb, C0, twoC0] (twoC0 = 0 for layer 0)
NCONST = 18

# pk1 (fp16, 1 partition): ones row | bhat1 | bhat2 | bhat3
PK1_ONES = 0
PK1_B = [8, 8 + H1, 8 + H1 + H2]
PK1_LEN = 8 + H1 + H2 + OUT

# pk8 (fp32, 8 partitions): x | m2r | m3r | m4r | cst | id8
PK8_X = 0
PK8_M = [None, IN, IN + H1]
PK8_M4 = IN + H1 + H2
PK8_CST = PK8_M4 + OUT
PK8_ID = PK8_CST + NCONST
PK8_LEN = PK8_ID + BL

# wall (fp16): xt | w1 chunks | w2 chunks | w3 chunks
XT_OFF = 0
XT_LEN = (IN // P128) * BL  # 64
W_OFF = [XT_LEN, XT_LEN + 4096, XT_LEN + 6144]
W_LEN = XT_LEN + 7168  # 7232

NKS = [IN // P128, H1 // P128, H2 // P128]
NOUTS = [H1, H2, OUT]

N_WARMUP = 10  # PE clock-gate warmup matmuls

_NC_CACHE = {}
DEBUG_TAPS = False


def _build_nc():
    import concourse.bacc as bacc
    import concourse.mybir as mybir
    import concourse.tile as tile

    fp32 = mybir.dt.float32
    fp16 = mybir.dt.float16
    AF = mybir.ActivationFunctionType
    ALU = mybir.AluOpType
    AX = mybir.AxisListType

    nc = bacc.Bacc("TRN2", target_bir_lowering=False, debug=False)

    pk1_t = nc.dram_tensor("pk1", [1, PK1_LEN], fp16, kind="ExternalInput")
    pk8_t = nc.dram_tensor("pk8", [BL, PK8_LEN], fp32, kind="ExternalInput")
    w_t = nc.dram_tensor("wall", [P128, W_LEN], fp16, kind="ExternalInput")
    out_t = nc.dram_tensor("outb", [BL, OUT], fp32, kind="ExternalOutput")

    with tile.TileContext(nc) as tc:
        with (
            tc.tile_pool(name="wp", bufs=1) as wp,
            tc.tile_pool(name="actp", bufs=1) as ap_,
            tc.tile_pool(name="scp", bufs=1) as scp,
            tc.tile_pool(name="pp", bufs=2, space="PSUM") as pp,
            tc.tile_pool(name="tpp", bufs=2, space="PSUM") as tpp,
        ):
            # --- PE warm-up: junk matmuls release the HAM clock gate ---
            junk_a = wp.tile([P128, BL], fp16, tag="junk_a")
            junk_w = wp.tile([P128, 512], fp16, tag="junk_w")
            nc.gpsimd.memset(junk_a[:], 0.0)
            nc.gpsimd.memset(junk_w[:], 0.0)
            warm_p = pp.tile([BL, 512], fp32, tag="warm")
            for _ in range(N_WARMUP):
                nc.tensor.matmul(
                    warm_p[:], junk_a[:, :BL], junk_w[:], start=True, stop=True
                )

            # --- DMAs (one serial completion chain per queue): wA leads so
            # L1 matmuls gate on the FIRST completion; pk8 second (stats are
            # needed mid-L1); pk1 after wB (bias row is the last L1 matmul).
            wseg = []  # (tile, col offset within wall)

            def wdma(name, lo, hi):
                t = wp.tile([P128, hi - lo], fp16, tag=name)
                nc.sync.dma_start(t[:], w_t[:, lo:hi])
                wseg.append((t, lo))

            wdma("wA", 0, 2624)           # xt + w1 chunks 0-4
            pk8 = ap_.tile([BL, PK8_LEN], fp32, tag="pk8")
            nc.sync.dma_start(pk8[:], pk8_t[:])
            wdma("wB", 2624, 4160)        # w1 chunks 5-7
            pk1 = ap_.tile([1, PK1_LEN], fp16, tag="pk1")
            nc.sync.dma_start(pk1[:], pk1_t[:])
            wdma("wC", 4160, 6208)        # w2
            wdma("wD", 6208, 7232)        # w3

            def wall_slice(lo, n):
                for t, off in wseg:
                    if off <= lo and lo + n <= off + t.shape[1]:
                        return t[:, lo - off : lo - off + n]
                raise AssertionError("bad wall slice")

            x_s = pk8[:, PK8_X : PK8_X + IN]
            id_s = pk8[:, PK8_ID : PK8_ID + BL]

            def col(j):
                c = PK8_CST + j
                return pk8[:, c : c + 1]

            # lhsT chunk slices per layer (fp16 [128, BL] each)
            vt = [[wall_slice(XT_OFF + k * BL, BL) for k in range(NKS[0])]]

            def layer(l, svec):
                """svec: {"sv": [BL,3] s|q tile, "ce": fused-delta operand}."""
                nk, nout = NKS[l], NOUTS[l]
                # alpha = C2*s + k_alpha
                al = scp.tile([BL, 1], fp32, tag=f"al{l}")
                nc.vector.tensor_scalar(
                    al[:], svec["sv"][:, 0:1], col(C_C2), col(C_KA0 + l),
                    ALU.mult, ALU.add
                )
                # delta = sum over crs_ext + k_delta.  For l>0, crs_ext is
                # [o*(2C0*m) | Cb*s | C0*q] (the cross block written by gpsimd
                # at the boundary); for l=0 only the [Cb*s | C0*q] tail exists.
                ce = svec["ce"]
                nc.vector.tensor_tensor(
                    ce[:, -2:],
                    svec["sv"][:, 0:2],
                    pk8[:, PK8_CST + C_TRI0 + 3 * l : PK8_CST + C_TRI0 + 3 * l + 2],
                    ALU.mult,
                )
                de = scp.tile([BL, 1], fp32, tag=f"de{l}")
                nc.vector.tensor_reduce(
                    out=de[:], in_=ce[:], axis=AX.X, op=ALU.add
                )
                de2 = scp.tile([BL, 1], fp32, tag=f"de2{l}")
                nc.vector.tensor_scalar(
                    de2[:], de[:], col(C_KD0 + l), None, ALU.add
                )
                # P = vi' @ W.T + bhat
                Pt = pp.tile([BL, nout], fp32, tag="P")
                for k in range(nk):
                    nc.tensor.matmul(
                        Pt[:],
                        vt[l][k],
                        wall_slice(W_OFF[l] + k * nout, nout),
                        start=(k == 0),
                        stop=False,
                    )
                boff = PK1_B[l]
                nc.tensor.matmul(
                    Pt[:],
                    pk1[:, PK1_ONES : PK1_ONES + BL],
                    pk1[:, boff : boff + nout],
                    start=False,
                    stop=True,
                )

                # epilogue: out' = relu(P*alpha) + (C1*P + delta)   [alpha > 0]
                vja = ap_.tile([BL, nout], fp32, tag=f"vja{l}")
                relu_inst = nc.scalar.activation(
                    out=vja[:], in_=Pt[:], func=AF.Relu, scale=al[:, 0:1],
                    bias=col(C_ZERO),
                )
                if l < 2:
                    # pinned keep-warm: ordered after the relu (sync=False =>
                    # no runtime wait) so the scheduler can't hoist them; they
                    # dispatch right after this layer's matmuls and keep the
                    # PE HAM clock-gate open through the epilogue gap
                    from concourse.tile_rust import add_dep_helper
                    for _ in range(8):
                        ji = nc.tensor.matmul(
                            warm_p[:], junk_a[:, :BL], junk_w[:],
                            start=True, stop=True,
                        )
                        add_dep_helper(
                            ji.ins, relu_inst.ins, sync=False,
                            reason="pin keep-warm after relu",
                        )
                tC = ap_.tile([BL, nout], fp32, tag=f"tC{l}")
                nc.vector.tensor_scalar(
                    tC[:], Pt[:], col(C_C1), de2[:, 0:1], ALU.mult, ALU.add
                )
                if l == 2:
                    # out = (vja + m4) + tC; the m4 add runs on gpsimd in
                    # parallel with tC on vector
                    gv = ap_.tile([BL, nout], fp32, tag="gv")
                    nc.gpsimd.tensor_tensor(
                        gv[:], vja[:], pk8[:, PK8_M4 : PK8_M4 + OUT], ALU.add
                    )
                    o = ap_.tile([BL, nout], fp32, tag=f"o{l}")
                    nc.vector.tensor_tensor(o[:], gv[:], tC[:], ALU.add)
                    return o, None
                o = ap_.tile([BL, nout], fp32, tag=f"o{l}")
                nc.vector.tensor_tensor(o[:], vja[:], tC[:], ALU.add)
                # transposes -> next layer's fp16 lhsT chunks (one copy)
                nch = nout // P128
                tp = tpp.tile([P128, nch * BL], fp32, tag="tp")
                for c in range(nch):
                    nc.tensor.transpose(
                        tp[:, c * BL : (c + 1) * BL],
                        o[:, c * P128 : (c + 1) * P128],
                        id_s,
                    )
                vtn = ap_.tile([P128, nch * BL], fp16, tag=f"vt{l + 1}")
                # explicit DVE: on ACT this copy queues behind the Square stat
                # (readiness order), delaying the next layer's matmuls.
                # Two half-copies so the next layer's first matmuls start
                # while the second half is still copying.
                h = (nch // 2) * BL
                nc.vector.tensor_copy(out=vtn[:, :h], in_=tp[:, :h])
                nc.vector.tensor_copy(out=vtn[:, h:], in_=tp[:, h:])
                vt.append([vtn[:, k * BL : (k + 1) * BL] for k in range(nch)])
                # next-layer stats; the cross products go on the idle
                # gpsimd engine straight into the fused-delta operand, and
                # Square (ACT) goes last so it cannot delay the transpose copy
                sv = scp.tile([BL, 3], fp32, tag=f"sv{l + 1}")
                nc.vector.reduce_sum(out=sv[:, 0:1], in_=o[:], axis=AX.X)
                ce = scp.tile([BL, nout + 2], fp32, tag=f"ce{l + 1}")
                nc.gpsimd.tensor_tensor(
                    ce[:, :nout], o[:],
                    pk8[:, PK8_M[l + 1] : PK8_M[l + 1] + nout], ALU.mult
                )
                sq = scp.tile([BL, nout], fp32, tag=f"sq{l + 1}")
                nc.scalar.activation(
                    out=sq[:], in_=o[:], func=AF.Square, bias=col(C_ZERO),
                    accum_out=sv[:, 1:2],
                )
                return o, {"sv": sv, "ce": ce[:, : nout + 2]}

            # layer-1 stats straight from fp32 x
            sv1 = scp.tile([BL, 3], fp32, tag="sv1")
            nc.vector.reduce_sum(out=sv1[:, 0:1], in_=x_s, axis=AX.X)
            sq0 = scp.tile([BL, IN], fp32, tag="sq0")
            nc.scalar.activation(
                out=sq0[:], in_=x_s, func=AF.Square, bias=col(C_ZERO),
                accum_out=sv1[:, 1:2],
            )
            ce1 = scp.tile([BL, 2], fp32, tag="ce1")

            o1, sv2 = layer(0, {"sv": sv1, "ce": ce1})
            o2, sv3 = layer(1, sv2)
            o3, _ = layer(2, sv3)

            nc.sync.dma_start(out_t[:], o3[:])

            if DEBUG_TAPS:
                for name, ap in (("dbg_o1", o1[:]), ("dbg_o2", o2[:])):
                    t = nc.dram_tensor(
                        name, list(ap.shape), ap.dtype, kind="ExternalOutput"
                    )
                    nc.sync.dma_start(t[:], ap)

    nc.compile()
    return nc


def get_nc():
    if "nc" not in _NC_CACHE:
        _NC_CACHE["nc"] = _build_nc()
    return _NC_CACHE["nc"]


def _chunk_pt(a, dtype):
    """[R, C] -> [128, (R//128)*C]: row-chunks of 128 side by side."""
    r, c = a.shape
    nk = r // P128
    return np.ascontiguousarray(
        a.reshape(nk, P128, c).transpose(1, 0, 2).reshape(P128, nk * c), dtype=dtype
    )


def host_prep(x, fc1_w, fc1_b, fc2_w, fc2_b, fc3_w, fc3_b,
              conv1_w, conv1_b, conv2_w, conv2_b, batch_num):
    f32, f16 = np.float32, np.float16
    x = np.asarray(x, f32)
    fc1_w = np.asarray(fc1_w, f32)
    fc2_w = np.asarray(fc2_w, f32)
    fc3_w = np.asarray(fc3_w, f32)
    fc1_b = np.asarray(fc1_b, f32)
    fc2_b = np.asarray(fc2_b, f32)
    fc3_b = np.asarray(fc3_b, f32)

    bn = float(np.asarray(batch_num).item())
    scale = RATE / bn
    coef = (np.asarray(conv2_w, np.float64) @ np.asarray(conv1_w, np.float64))[0]
    bc = float(
        (np.asarray(conv2_w, np.float64) @ np.asarray(conv1_b, np.float64))[0]
        + np.asarray(conv2_b, np.float64)[0]
    )
    C0, C1, C2 = (scale * coef).astype(np.float64)
    Cb = scale * bc

    m2 = (-C1 * fc1_b.astype(np.float64)).astype(f32)
    m3 = (-C1 * fc2_b.astype(np.float64)).astype(f32)
    m4 = (-C1 * fc3_b.astype(np.float64)).astype(f32)
    bh1 = fc1_b
    bh2 = (fc2_b + m2 @ fc2_w.T).astype(f32)
    bh3 = (fc3_b + m3 @ fc3_w.T).astype(f32)

    ka = [1.0, 1.0 + C2 * float(m2.sum()), 1.0 + C2 * float(m3.sum())]
    kd = [
        0.0,
        C0 * float(m2 @ m2) + Cb * float(m2.sum()),
        C0 * float(m3 @ m3) + Cb * float(m3.sum()),
    ]
    cvec = np.zeros(NCONST, dtype=f32)
    cvec[C_C1], cvec[C_C2] = C1, C2
    cvec[C_KA0 : C_KA0 + 3] = ka
    cvec[C_KD0 : C_KD0 + 3] = kd
    for l in range(3):
        cvec[C_TRI0 + 3 * l : C_TRI0 + 3 * l + 3] = [
            Cb, C0, 0.0 if l == 0 else 2 * C0
        ]

    pk1 = np.zeros((1, PK1_LEN), f16)
    pk1[0, PK1_ONES : PK1_ONES + BL] = 1.0
    pk1[0, PK1_B[0] : PK1_B[0] + H1] = bh1.astype(f16)
    pk1[0, PK1_B[1] : PK1_B[1] + H2] = bh2.astype(f16)
    pk1[0, PK1_B[2] : PK1_B[2] + OUT] = bh3.astype(f16)

    wall_base = np.empty((P128, W_LEN), f16)
    wall_base[:, W_OFF[0] : W_OFF[0] + 4096] = _chunk_pt(fc1_w.T, f16)
    wall_base[:, W_OFF[1] : W_OFF[1] + 2048] = _chunk_pt(fc2_w.T, f16)
    wall_base[:, W_OFF[2] : W_OFF[2] + 1024] = _chunk_pt(fc3_w.T, f16)

    pk8_base = np.zeros((BL, PK8_LEN), f32)
    pk8_base[:, PK8_M[1] : PK8_M[1] + H1] = (2.0 * C0 * m2.astype(np.float64)).astype(f32)
    pk8_base[:, PK8_M[2] : PK8_M[2] + H2] = (2.0 * C0 * m3.astype(np.float64)).astype(f32)
    pk8_base[:, PK8_M4 : PK8_M4 + OUT] = m4
    pk8_base[:, PK8_CST : PK8_CST + NCONST] = cvec
    pk8_base[:, PK8_ID : PK8_ID + BL] = np.eye(BL, dtype=f32)

    in_maps = []
    for k in range(NCORES):
        xk = np.ascontiguousarray(x[k * BL : (k + 1) * BL], dtype=f32)
        pk8 = pk8_base.copy()
        pk8[:, PK8_X : PK8_X + IN] = xk
        wall = wall_base.copy()
        wall[:, XT_OFF : XT_OFF + XT_LEN] = _chunk_pt(xk.T.copy(), f16)
        in_maps.append({"pk1": pk1, "pk8": pk8, "wall": wall})
    return in_maps


def kernel(**inputs):
    from concourse.bass_utils import run_bass_kernel_spmd

    nc = get_nc()
    in_maps = host_prep(**inputs)
    res = run_bass_kernel_spmd(nc, in_maps, core_ids=list(range(NCORES)))
    out = np.concatenate([res.results[k]["outb"] for k in range(NCORES)], axis=0)
    return np.ascontiguousarray(out, dtype=np.float32)
